# revision 7
# baseline (speedup 1.0000x reference)
# BitLinear (eval path) Trainium2 kernel: ternary weight quant + int8 activation
# quant + dense matmul, tensor-parallel over 8 NeuronCores.
#
# Math (per reference):
#   w_scale[o] = max(mean_k |W[o,k]|, EPS)
#   w_quant    = clip(round(W / w_scale), -1, 1)            (ternary)
#   x_scale[t] = max(max_k |x[t,k]| / 127, EPS)
#   x_quant    = round(x / x_scale)                          (int8 range)
#   out[t,o]   = (sum_k x_quant[t,k] * w_quant[o,k]) * x_scale[t] * w_scale[o] + bias[o]
#
# The integer sum is computed exactly on the PE: w_quant is exact in fp8e4,
# x_quant (|v| <= 127) is exact in bf16, products/partials are exact in the
# fp32 PSUM accumulator (max |sum| <= 127*4096 < 2^24).
#
# Sharding: 4 token groups x 2 out-feature groups = 8 cores. Host passes
# transposed (K-major) layouts so both matmul operands stream with K on
# partitions; all arithmetic happens on-device.
import numpy as np

import concourse.bacc as bacc
import concourse.bass as bass
import concourse.tile as tile
from concourse import mybir
from concourse.bass_utils import run_bass_kernel_spmd

F32 = mybir.dt.float32
BF16 = mybir.dt.bfloat16
FP8 = mybir.dt.float8e4

EPS = 1e-5
MAGIC = 12582912.0  # 1.5 * 2^23: (x + MAGIC) - MAGIC == rint(x) for |x| < 2^22

# Full-problem shapes (hardcoded per contract).
B, S, I, O = 4, 2048, 4096, 4096
T_FULL = B * S  # 8192 tokens
TSPLIT, OSPLIT = 4, 2  # token groups x out-feature groups = 8 cores
N_CORES = TSPLIT * OSPLIT


def build_nc(K=I, TO=O // OSPLIT, TT=T_FULL // TSPLIT, OB=256, TCH=256):
    """Build the per-core Bass program. Every core runs the same program on
    its own shard: xT [K, TT], wT [K, TO], bias [TO] -> outT [TO, TT]."""
    KT = K // 128  # k subtiles
    NOB = TO // OB  # weight column blocks
    NOT = TO // 128  # output row tiles
    NCH = TT // TCH  # token chunks
    OB_T = OB // 128  # out tiles per weight block

    nc = bacc.Bacc("TRN2", target_bir_lowering=False, debug=False)
    xT = nc.dram_tensor("xT", [K, TT], F32, kind="ExternalInput").ap()
    wT = nc.dram_tensor("wT", [K, TO], F32, kind="ExternalInput").ap()
    bias_d = nc.dram_tensor("bias", [TO], F32, kind="ExternalInput").ap()
    outT = nc.dram_tensor("outT", [TO, TT], F32, kind="ExternalOutput").ap()

    # K-major DRAM views: [p, kt, cols]
    x_v = xT.rearrange("(kt p) t -> p kt t", p=128)
    w_v = wT.rearrange("(kt p) o -> p kt o", p=128)
    bias_v = bias_d.rearrange("(ot p) -> p ot", p=128)

    with tile.TileContext(nc) as tc:
        with (
            tc.tile_pool(name="blk", bufs=2) as p_blk,  # f32 input blocks (shared W/x)
            tc.tile_pool(name="wq", bufs=1) as p_wq,
            tc.tile_pool(name="xq", bufs=2) as p_xq,
            tc.tile_pool(name="small", bufs=3) as p_small,  # abs subtiles
            tc.tile_pool(name="bc", bufs=2) as p_bc,  # broadcast tiles
            tc.tile_pool(name="rows", bufs=2) as p_rows,
            tc.tile_pool(name="amax", bufs=2) as p_amax,
            tc.tile_pool(name="osb", bufs=3) as p_osb,
            tc.tile_pool(name="osb2", bufs=3) as p_osb2,
            tc.tile_pool(name="const", bufs=1) as p_const,
            tc.tile_pool(name="ps_mm", bufs=2, space="PSUM") as ps_mm,
            tc.tile_pool(name="ps_ws", bufs=2, space="PSUM") as ps_ws,
            tc.tile_pool(name="ps_bc", bufs=2, space="PSUM") as ps_bc,
        ):
            ones_k = p_const.tile([128, 1], F32)
            nc.vector.memset(ones_k[:], 1.0)
            ones_r = p_const.tile([1, 128], F32)
            nc.vector.memset(ones_r[:], 1.0)
            one_1 = p_const.tile([1, 1], F32)
            nc.vector.memset(one_1[:], 1.0)
            from concourse.masks import make_identity

            ident128 = p_const.tile([128, 128], F32)
            make_identity(nc, ident128[:])

            # Resident quantized weights [p, kt, o] fp8 and per-o-tile scales.
            wq_all = p_wq.tile([128, KT, TO], FP8)
            ws_col = p_const.tile([128, NOT], F32)
            bias_col = p_const.tile([128, NOT], F32)
            nc.sync.dma_start(out=bias_col[:, 0:NOT], in_=bias_v)

            def bcast_row(row_ap, width):
                """[1, width] sbuf row -> [128, width] sbuf tile via PE."""
                ps = ps_bc.tile([128, width], F32, tag="bc")
                nc.tensor.matmul(
                    ps[:], ones_r[:, 0:128], row_ap, start=True, stop=True
                )
                sb = p_bc.tile([128, width], F32, tag="bcsb")
                nc.vector.tensor_copy(sb[:], ps[:])
                return sb

            # ---------------- W phase: scales + ternary quantization ----------
            for ob in range(NOB):
                w_blk = p_blk.tile([128, KT, OB], F32, tag="blk")
                nc.sync.dma_start(
                    out=w_blk[:], in_=w_v[:, :, ob * OB : (ob + 1) * OB]
                )
                # sum_k |W[k, o]| via ACT abs + PE ones-matmul (reduces both
                # the partition dim and the kt dim into one psum row).
                pws = ps_ws.tile([1, OB], F32, tag="ws")
                for kt in range(KT):
                    a_s = p_small.tile([128, OB], F32, tag="abs")
                    nc.scalar.activation(
                        out=a_s[:],
                        in_=w_blk[:, kt, :],
                        func=mybir.ActivationFunctionType.Abs,
                    )
                    nc.tensor.matmul(
                        pws[:],
                        ones_k[:],
                        a_s[:],
                        start=(kt == 0),
                        stop=(kt == KT - 1),
                    )
                # w_scale = max(sum/K, EPS); r = 1/w_scale
                ws_row = p_rows.tile([1, OB], F32, tag="wsrow")
                nc.vector.tensor_scalar(
                    out=ws_row[:],
                    in0=pws[:],
                    scalar1=1.0 / K,
                    scalar2=EPS,
                    op0=mybir.AluOpType.mult,
                    op1=mybir.AluOpType.max,
                )
                rws_row = p_rows.tile([1, OB], F32, tag="rwsrow")
                nc.vector.reciprocal(rws_row[:], ws_row[:])
                rws_bc = bcast_row(rws_row[:], OB)
                # broadcast over kt via 0-step AP
                rws_bc_kt = bass.AP(
                    tensor=rws_bc.tensor,
                    offset=rws_bc.offset,
                    ap=[rws_bc.ap[0], [0, KT], rws_bc.ap[1]],
                )
                # w *= r  (in place, f32)
                nc.vector.tensor_tensor(
                    out=w_blk[:], in0=w_blk[:], in1=rws_bc_kt, op=mybir.AluOpType.mult
                )
                # round to nearest (exact, RNE)
                nc.vector.tensor_scalar(
                    out=w_blk[:],
                    in0=w_blk[:],
                    scalar1=MAGIC,
                    scalar2=MAGIC,
                    op0=mybir.AluOpType.add,
                    op1=mybir.AluOpType.subtract,
                )
                # clip to {-1,0,1} and store as fp8
                nc.vector.tensor_scalar(
                    out=wq_all[:, :, ob * OB : (ob + 1) * OB],
                    in0=w_blk[:],
                    scalar1=1.0,
                    scalar2=-1.0,
                    op0=mybir.AluOpType.min,
                    op1=mybir.AluOpType.max,
                )
                # ws row -> per-partition columns of ws_col (PE transpose trick)
                for c in range(OB_T):
                    ot = ob * OB_T + c
                    pcol = ps_bc.tile([128, 1], F32, tag="bc")
                    nc.tensor.matmul(
                        pcol[:],
                        ws_row[0:1, c * 128 : (c + 1) * 128],
                        one_1[:],
                        start=True,
                        stop=True,
                    )
                    nc.vector.tensor_copy(ws_col[:, ot : ot + 1], pcol[:])

            # ---------------- main loop: x quant + matmul + epilogue ----------
            for ch in range(NCH):
                x_blk = p_blk.tile([128, KT, TCH], F32, tag="blk")
                nc.sync.dma_start(
                    out=x_blk[:], in_=x_v[:, :, ch * TCH : (ch + 1) * TCH]
                )
                # amax over kt (innermost via transposed view), |.| applied
                am = p_amax.tile([128, TCH], F32, tag="amax")
                x_tkt = x_blk[:].rearrange("p kt t -> p t kt")
                nc.vector.tensor_reduce(
                    out=am[:],
                    in_=x_tkt,
                    axis=mybir.AxisListType.X,
                    op=mybir.AluOpType.max,
                    apply_absolute_value=True,
                )
                # partition-dim max via PE transpose + free-dim reduce
                # (DVE cannot combine operands at different base partitions).
                am_row = p_rows.tile([1, TCH], F32, tag="amrow")
                for j in range(TCH // 128):
                    pT = ps_bc.tile([128, 128], F32, tag="bc")
                    nc.tensor.transpose(
                        pT[:], am[:, j * 128 : (j + 1) * 128], ident128[:]
                    )
                    col = p_rows.tile([128, 1], F32, tag="amcol")
                    nc.vector.tensor_reduce(
                        out=col[:],
                        in_=pT[:],
                        axis=mybir.AxisListType.X,
                        op=mybir.AluOpType.max,
                    )
                    prow = ps_bc.tile([1, 128], F32, tag="bc")
                    nc.tensor.transpose(prow[:], col[:], ident128[:])
                    nc.vector.tensor_copy(
                        am_row[0:1, j * 128 : (j + 1) * 128], prow[:]
                    )
                xs_row = p_rows.tile([1, TCH], F32, tag="xsrow")
                nc.vector.tensor_scalar(
                    out=xs_row[:],
                    in0=am_row[:],
                    scalar1=1.0 / 127.0,
                    scalar2=EPS,
                    op0=mybir.AluOpType.mult,
                    op1=mybir.AluOpType.max,
                )
                rxs_row = p_rows.tile([1, TCH], F32, tag="rxsrow")
                nc.vector.reciprocal(rxs_row[:], xs_row[:])
                rxs_bc = bcast_row(rxs_row[:], TCH)
                xs_bc = bcast_row(xs_row[:], TCH)
                rxs_bc_kt = bass.AP(
                    tensor=rxs_bc.tensor,
                    offset=rxs_bc.offset,
                    ap=[rxs_bc.ap[0], [0, KT], rxs_bc.ap[1]],
                )
                nc.vector.tensor_tensor(
                    out=x_blk[:], in0=x_blk[:], in1=rxs_bc_kt, op=mybir.AluOpType.mult
                )
                xq = p_xq.tile([128, KT, TCH], BF16, tag="xq")
                nc.vector.tensor_scalar(
                    out=xq[:],
                    in0=x_blk[:],
                    scalar1=MAGIC,
                    scalar2=MAGIC,
                    op0=mybir.AluOpType.add,
                    op1=mybir.AluOpType.subtract,
                )
                for ot in range(NOT):
                    pmm = ps_mm.tile([128, TCH], F32, tag="mm")
                    for kt in range(KT):
                        nc.tensor.matmul(
                            pmm[:],
                            wq_all[:, kt, ot * 128 : (ot + 1) * 128],
                            xq[:, kt, :],
                            start=(kt == 0),
                            stop=(kt == KT - 1),
                        )
                    # (psum * ws[o]) * xs[t]
                    osb = p_osb.tile([128, TCH], F32, tag="osb")
                    nc.vector.scalar_tensor_tensor(
                        out=osb[:],
                        in0=pmm[:],
                        scalar=ws_col[:, ot : ot + 1],
                        in1=xs_bc[:],
                        op0=mybir.AluOpType.mult,
                        op1=mybir.AluOpType.mult,
                    )
                    # + bias[o]
                    osb2 = p_osb2.tile([128, TCH], F32, tag="osb2")
                    nc.scalar.activation(
                        out=osb2[:],
                        in_=osb[:],
                        func=mybir.ActivationFunctionType.Identity,
                        bias=bias_col[:, ot : ot + 1],
                        scale=1.0,
                    )
                    nc.sync.dma_start(
                        out=outT[
                            ot * 128 : (ot + 1) * 128, ch * TCH : (ch + 1) * TCH
                        ],
                        in_=osb2[:],
                    )
    nc.compile()
    return nc


_NC_CACHE = {}
TRACE = False
LAST_EXEC_NS = None


def _get_nc():
    key = "full"
    if key not in _NC_CACHE:
        _NC_CACHE[key] = build_nc()
    return _NC_CACHE[key]


def _run(x, weight, bias, trace=False):
    global LAST_EXEC_NS
    x = np.asarray(x, dtype=np.float32)
    weight = np.asarray(weight, dtype=np.float32)
    bias = np.asarray(bias, dtype=np.float32)

    xT = np.ascontiguousarray(x.reshape(T_FULL, I).T)  # [I, T]
    wT = np.ascontiguousarray(weight.T)  # [I, O]

    TT = T_FULL // TSPLIT
    TO = O // OSPLIT
    in_maps = []
    for c in range(N_CORES):
        ti, oj = divmod(c, OSPLIT)
        in_maps.append(
            {
                "xT": np.ascontiguousarray(xT[:, ti * TT : (ti + 1) * TT]),
                "wT": np.ascontiguousarray(wT[:, oj * TO : (oj + 1) * TO]),
                "bias": np.ascontiguousarray(bias[oj * TO : (oj + 1) * TO]),
            }
        )

    nc = _get_nc()
    res = run_bass_kernel_spmd(
        nc, in_maps, core_ids=list(range(N_CORES)), trace=trace
    )
    LAST_EXEC_NS = res.exec_time_ns

    out = np.empty((T_FULL, O), dtype=np.float32)
    for c in range(N_CORES):
        ti, oj = divmod(c, OSPLIT)
        out[ti * TT : (ti + 1) * TT, oj * TO : (oj + 1) * TO] = res.results[c][
            "outT"
        ].T
    return out.reshape(B, S, O)


def kernel(x, weight, bias):
    return _run(x, weight, bias, trace=False)


def kernel_traced(x, weight, bias):
    _run(x, weight, bias, trace=True)
    return LAST_EXEC_NS
